# revision 1
# baseline (speedup 1.0000x reference)
"""SO3Conv Trainium2 Bass kernel.

Math (per reference):
  psi[f,g,i] = sum_n D[n,i] w[f,g,n] / sqrt(64)
  per l (d=2l+1, blk=d*d at offset off):
    y[b,g,off+v*d+m] = 1/sqrt(64*d) * sum_{f,u} x[b,f,off+u*d+m] * psi[f,g,off+u*d+v]

Strategy: data-parallel over batch (8 cores x 128 batch).
Per core, all matmul operands in bf16 (fp32 PSUM accumulate):
  A) psi computed on PE in "psiT" layout [(u,v)-part, (f,g)-free], then
     reshaped via SBUF->SBUF DMA into per-(l,ku) rhs tiles
     [(u,f)-part, (v,g)-free]  (K-chunks ku = pairs of u, 2*64=128 rows).
  B) x loaded contiguously (SWDGE cast fp32->bf16), transposed on PE per
     (l, ku, m) into lhsT tiles [(u,f)-part, b-free].
  C) matmuls accumulate over ku into PSUM [b, (v,g)], copied (cast bf16,
     scattered) into full y in natural layout, stored with SWDGE cast
     bf16->fp32.
"""

import sys

sys.path.insert(0, "/opt/trn_rl_repo")

import numpy as np

LMAX = 6
F = 64
NROT = 64
IRREP = 455
B = 1024
NCORES = 8
BS = B // NCORES  # 128

DS = [2 * l + 1 for l in range(LMAX + 1)]
OFFS = []
_o = 0
for _d in DS:
    OFFS.append(_o)
    _o += _d * _d
assert _o == IRREP

_CACHE = {}


def _build():
    import concourse.bacc as bacc
    import concourse.bass as bass
    import concourse.mybir as mybir
    from concourse import tile

    dt = mybir.dt
    BF = dt.bfloat16
    F32 = dt.float32

    nc = bacc.Bacc("TRN2", target_bir_lowering=False, debug=False, num_devices=NCORES)

    x_d = nc.dram_tensor("x", [BS, F, IRREP], F32, kind="ExternalInput")
    D_d = nc.dram_tensor("D", [NROT, IRREP], F32, kind="ExternalInput")
    w_d = nc.dram_tensor("w", [F, F, NROT], F32, kind="ExternalInput")
    id_d = nc.dram_tensor("ident", [128, 128], BF, kind="ExternalInput")
    y_d = nc.dram_tensor("y", [BS, F, IRREP], F32, kind="ExternalOutput")
    # DRAM scratch for the psi layout shuffle: S[i, (f,g)]
    s_d = nc.dram_tensor("psiS", [IRREP, F * F], BF)

    with tile.TileContext(nc) as tc:
        with (
            tc.tile_pool(name="big", bufs=1) as big,
            tc.tile_pool(name="rhs", bufs=1) as rhsp,
            tc.tile_pool(name="const", bufs=1) as cp,
            tc.tile_pool(name="pt", bufs=2, space=bass.MemorySpace.PSUM) as pt,
            tc.tile_pool(name="py", bufs=4, space=bass.MemorySpace.PSUM) as py,
        ):
            # ---- persistent SBUF ----
            x_bf = big.tile([BS, F, IRREP], BF)
            y_bf = big.tile([BS, F, IRREP], BF)
            ident = cp.tile([128, 128], BF)
            nc.sync.dma_start(ident[:, :], id_d[:, :])

            # rhs tiles per (l, ku):  [krows, d*64] bf16, free idx = v*64+g
            rhs = {}
            for l in range(LMAX + 1):
                d = DS[l]
                nku = (d + 1) // 2
                for ku in range(nku):
                    nu = 2 if (ku * 2 + 1) < d else 1
                    rhs[(l, ku)] = rhsp.tile([nu * 64, d * 64], BF, name=f"rhs{l}_{ku}", tag=f"rhs{l}_{ku}")

            # ---- load D (scaled 1/8, bf16) and w (bf16) ----
            d_f32 = cp.tile([NROT, IRREP], F32)
            nc.sync.dma_start(d_f32[:, :], D_d[:, :])
            d_bf = cp.tile([NROT, IRREP], BF)
            nc.scalar.mul(d_bf[:, :], d_f32[:, :], 1.0 / 8.0)

            # w (f,g,n) -> w_bf [128, 32, 64] : partition p, chunk c of (f*64+g)=c*128+p
            w_bf = cp.tile([128, 32, NROT], BF)
            w_view = w_d.rearrange("f g n -> (f g) n").rearrange(
                "(c p) n -> p c n", p=128
            )
            nc.gpsimd.dma_start(w_bf[:, :, :], w_view)

            # ---- x load (SWDGE cast): l=6 now; the rest after psi phase ----
            _mid6 = (OFFS[6] + IRREP) // 2
            _mid5 = (OFFS[5] + OFFS[6]) // 2
            for i0, i1 in ((OFFS[6], _mid6), (_mid6, IRREP)):
                nc.gpsimd.dma_start(x_bf[:, :, i0:i1], x_d[:, :, i0:i1])

            # wT [n=64, (f,g)=4096] via PE transposes
            wT = cp.tile([NROT, F * F], BF)
            for cgrp in range(4):  # 8 transposes per psum bank
                ps = pt.tile([128, 1024], BF, tag="ptx", name="psw")
                for t in range(8):
                    c = cgrp * 8 + t
                    nc.tensor.transpose(
                        ps[:64, t * 128 : (t + 1) * 128], w_bf[:, c, :], ident[:, :]
                    )
                nc.vector.tensor_copy(
                    wT[:, cgrp * 1024 : (cgrp + 1) * 1024], ps[:64, :]
                )

            # ---- psi in psiT layout + reshape to rhs tiles ----
            # psiT chunk rows r = flat (u*d+v) index within l-block (<=128 rows)
            s_fvg = s_d.rearrange("i (f g) -> f i g", g=64)
            with (
                tc.tile_pool(name="lhs", bufs=1) as lp,
                tc.tile_pool(name="psit", bufs=2) as psp,
                tc.tile_pool(name="pa", bufs=2, space=bass.MemorySpace.PSUM) as pa,
            ):
                eng_flip = 0
                for l in range(LMAX, -1, -1):
                    d = DS[l]
                    blk = d * d
                    off = OFFS[l]
                    norm = 1.0 / np.sqrt(64.0 * d)
                    r0 = 0
                    while r0 < blk:
                        rows = min(128, blk - r0)
                        psiT = psp.tile([128, F * F], BF, tag="psiT")
                        for s in range(8):
                            pps = pa.tile([128, 512], F32, tag="ptp", name="pps")
                            nc.tensor.matmul(
                                pps[:rows, :],
                                d_bf[:, off + r0 : off + r0 + rows],
                                wT[:, s * 512 : (s + 1) * 512],
                                start=True,
                                stop=True,
                            )
                            dst = psiT[:rows, s * 512 : (s + 1) * 512]
                            if eng_flip % 2 == 0:
                                nc.scalar.mul(dst, pps[:rows, :], norm)
                            else:
                                nc.vector.tensor_scalar_mul(dst, pps[:rows, :], norm)
                            eng_flip += 1
                        # park this chunk in DRAM scratch (contiguous rows)
                        nc.sync.dma_start(
                            s_d[off + r0 : off + r0 + rows, :], psiT[:rows, :]
                        )
                        r0 += rows
                    # read back with (f, v, g)-ordered APs into rhs tiles
                    for u in range(d):
                        ku, uin = divmod(u, 2)
                        src_ap = s_fvg[:, off + u * d : off + (u + 1) * d, :]
                        dst = rhs[(l, ku)][uin * 64 : (uin + 1) * 64, :].rearrange(
                            "f (v g) -> f v g", g=64
                        )
                        nc.sync.dma_start(dst, src_ap)

                # ---- rest of x (after psi DMAs in priority order) ----
                for i0, i1 in (
                    (OFFS[5], _mid5),
                    (_mid5, OFFS[6]),
                    (OFFS[4], OFFS[5]),
                    (OFFS[3], OFFS[4]),
                    (0, OFFS[3]),
                ):
                    nc.gpsimd.dma_start(x_bf[:, :, i0:i1], x_d[:, :, i0:i1])

                # ---- main loop ----
                for l in range(LMAX, -1, -1):
                    d = DS[l]
                    off = OFFS[l]
                    nku = (d + 1) // 2
                    if d * 64 <= 512:
                        vsplits = [(0, d)]
                    else:
                        vh = (d + 1) // 2
                        vsplits = [(0, vh), (vh, d - vh)]

                    xv = x_bf[:, :, off : off + d * d].rearrange(
                        "b f (u m) -> b u f m", u=d
                    )
                    lts = []
                    for ku in range(nku):
                        nu = 2 if (ku * 2 + 1) < d else 1
                        lt = lp.tile(
                            [nu * 64, d * 128], BF, tag=f"lhsT{ku}", name=f"lt{l}_{ku}"
                        )
                        lts.append(lt)
                        for m0 in range(0, d, 8):
                            mm = min(8, d - m0)
                            ps = pt.tile([128, 1024], BF, tag="ptx", name="psx")
                            for t in range(mm):
                                m = m0 + t
                                for uin in range(nu):
                                    src = xv[:, 2 * ku + uin, :, m]
                                    nc.tensor.transpose(
                                        ps[
                                            uin * 64 : (uin + 1) * 64,
                                            t * 128 : (t + 1) * 128,
                                        ],
                                        src,
                                        ident[:, :],
                                    )
                            nc.vector.tensor_copy(
                                lt[:, m0 * 128 : (m0 + mm) * 128],
                                ps[: nu * 64, : mm * 128],
                            )
                    yv = y_bf[:, :, off : off + d * d].rearrange(
                        "b g (v m) -> b v g m", v=d
                    )
                    for m in range(d):
                        for v0, nv in vsplits:
                            pyt = py.tile([BS, 512], F32, tag="py", name="pyt")
                            out = pyt[:, : nv * 64]
                            for ku in range(nku):
                                nc.tensor.matmul(
                                    out,
                                    lts[ku][:, m * 128 : (m + 1) * 128],
                                    rhs[(l, ku)][:, v0 * 64 : (v0 + nv) * 64],
                                    start=(ku == 0),
                                    stop=(ku == nku - 1),
                                )
                            dst = yv[:, v0 : v0 + nv, :, m]
                            src = out.rearrange("b (v g) -> b v g", g=64)
                            if (m + v0) % 2 == 0:
                                nc.scalar.copy(dst, src)
                            else:
                                nc.vector.tensor_copy(dst, src)

            # ---- store y (SWDGE cast bf16->fp32), 4 col-chunks ----
            yflat_s = y_bf.rearrange("b f i -> b (f i)")
            yflat_d = y_d.rearrange("b f i -> b (f i)")
            CH = F * IRREP // 4
            for c in range(4):
                nc.gpsimd.dma_start(
                    yflat_d[:, c * CH : (c + 1) * CH], yflat_s[:, c * CH : (c + 1) * CH]
                )

    nc.compile()
    return nc


def _get_nc():
    if "nc" not in _CACHE:
        _CACHE["nc"] = _build()
    return _CACHE["nc"]


def kernel(x, D, w):
    import ml_dtypes
    from concourse.bass_utils import run_bass_kernel_spmd

    nc = _get_nc()
    ident = np.eye(128, dtype=ml_dtypes.bfloat16)
    in_maps = [
        {
            "x": np.ascontiguousarray(x[c * BS : (c + 1) * BS]),
            "D": np.ascontiguousarray(D),
            "w": np.ascontiguousarray(w),
            "ident": ident,
        }
        for c in range(NCORES)
    ]
    res = run_bass_kernel_spmd(nc, in_maps, core_ids=list(range(NCORES)))
    out = np.concatenate([r["y"] for r in res.results], axis=0)
    return out.astype(np.float32)



# revision 2
# speedup vs baseline: 1.1856x; 1.1856x over previous
"""SO3Conv Trainium2 Bass kernel.

Math (per reference):
  psi[f,g,i] = sum_n D[n,i] w[f,g,n] / sqrt(64)
  per l (d=2l+1, blk=d*d at offset off):
    y[b,g,off+v*d+m] = 1/sqrt(64*d) * sum_{f,u} x[b,f,off+u*d+m] * psi[f,g,off+u*d+v]

Strategy: data-parallel over batch (8 cores x 128 batch).
Per core:
  A) x is pre-permuted on the host into per-l regions [b, (m, u-pad, f)]
     bf16 (u padded to d+1 slots).  XBAR DMA-transposes (InstDmaTransposeAnt)
     then produce the matmul lhsT tiles [(u,f)-part, b-free] directly from
     DRAM -- no SBUF staging, no PE transposes.
  B) wT [n, (f g)] likewise via one XBAR from host-padded w2 [(f g), n-pad].
  C) psi computed on PE in psiT layout [i-chunk-part, (f g)-free] (D
     pre-scaled per l so no later normalization), parked in DRAM (5
     chunk-aligned scratch tensors), read back with (f,v,g) APs into
     per-(l,ku) rhs tiles [(u-pair,f)-part, (v,g)-free].
  D) main matmuls accumulate over ku into PSUM [b, (v g)] fp32 (K=64 for the
     odd-u tail), copied (cast bf16) into per-l y tiles in natural
     [b, g, v*d+m] order, stored bf16 to per-l DRAM regions; host converts
     to fp32 and reassembles.
"""

import sys

sys.path.insert(0, "/opt/trn_rl_repo")

import numpy as np

LMAX = 6
F = 64
NROT = 64
IRREP = 455
B = 1024
NCORES = 8
BS = B // NCORES  # 128

DS = [2 * l + 1 for l in range(LMAX + 1)]
OFFS = []
_o = 0
for _d in DS:
    OFFS.append(_o)
    _o += _d * _d
assert _o == IRREP

LORDER = list(range(LMAX, -1, -1))  # process l descending

# x4 DRAM region offsets (l descending), cols per l = d*(d+1)*64
XLEN = {l: DS[l] * (DS[l] + 1) * 64 for l in LORDER}
XOFF = {}
_o = 0
for l in LORDER:
    XOFF[l] = _o
    _o += XLEN[l]
XTOT = _o  # 32256

# y DRAM region offsets (l descending), cols per l = 64*blk
YLEN = {l: 64 * DS[l] * DS[l] for l in LORDER}
YOFF = {}
_o = 0
for l in LORDER:
    YOFF[l] = _o
    _o += YLEN[l]
YTOT = _o  # 29120

# psi chunks: contiguous i-ranges aligned to l- and u-boundaries, <=128 rows,
# ordered so l=6 is computed first.
PSI_CHUNKS = [
    (OFFS[6], OFFS[6] + 9 * 13),       # l6 u0..8   (117 rows)
    (OFFS[6] + 9 * 13, IRREP),         # l6 u9..12  (52 rows)
    (OFFS[5], OFFS[6]),                # l5         (121 rows)
    (OFFS[4], OFFS[5]),                # l4         (81 rows)
    (0, OFFS[4]),                      # l0..l3     (84 rows)
]

_CACHE = {}


def _build():
    import concourse.bacc as bacc
    import concourse.bass as bass
    import concourse.mybir as mybir
    from concourse import tile

    dt = mybir.dt
    BF = dt.bfloat16
    F32 = dt.float32

    nc = bacc.Bacc("TRN2", target_bir_lowering=False, debug=False, num_devices=NCORES)

    x_d = nc.dram_tensor("x4", [BS, XTOT], BF, kind="ExternalInput")
    w_d = nc.dram_tensor("w2", [F * F, 128], BF, kind="ExternalInput")
    D_d = nc.dram_tensor("D", [NROT, IRREP], F32, kind="ExternalInput")
    y_d = nc.dram_tensor("y", [BS, YTOT], BF, kind="ExternalOutput")
    # psi DRAM parking: one scratch tensor per chunk (exact DMA deps)
    s_d = [
        nc.dram_tensor(f"psiS{ci}", [r1 - r0, F * F], BF)
        for ci, (r0, r1) in enumerate(PSI_CHUNKS)
    ]

    with tile.TileContext(nc) as tc:
        with (
            tc.tile_pool(name="const", bufs=1) as cp,
            tc.tile_pool(name="xt", bufs=1) as xp,
            tc.tile_pool(name="rhs", bufs=1) as rp,
            tc.tile_pool(name="yb", bufs=1) as yp,
            tc.tile_pool(name="psit", bufs=2) as psp,
            tc.tile_pool(name="pa", bufs=2, space=bass.MemorySpace.PSUM) as pa,
            tc.tile_pool(name="py", bufs=4, space=bass.MemorySpace.PSUM) as py,
        ):
            # ---- persistent tiles ----
            wT = cp.tile([128, F * F], BF)  # rows 0:64 = n, rest pad
            d_f32 = cp.tile([NROT, IRREP], F32)
            d_pre = cp.tile([NROT, IRREP], BF)
            xt = {}   # per l: [128, nchunk, 128]; chunk c = m*(d+1)/2 + ku
            rhs = {}  # per l: [128, nku*d*64]; ku slab cols [ku*d*64, ...)
            yb = {}   # per l: [128, 64*blk]
            for l in LORDER:
                d = DS[l]
                nch = d * (d + 1) // 2
                xt[l] = xp.tile([128, nch, 128], BF, name=f"xt{l}", tag=f"xt{l}")
                rhs[l] = rp.tile(
                    [128, ((d + 1) // 2) * d * 64], BF, name=f"rhs{l}", tag=f"rhs{l}"
                )
                yb[l] = yp.tile([BS, 64 * d * d], BF, name=f"yb{l}", tag=f"yb{l}")

            # ---- scalar-queue DMAs: D load, wT XBAR ----
            nc.scalar.dma_start(d_f32[:, :], D_d[:, :])
            nc.scalar.dma_start(wT[:, :], w_d[:, :], transpose=True)
            # fold both norms into D: scale_l = 1/(8*sqrt(64*d)) = 1/(64*sqrt(d))
            for l in LORDER:
                off, blk = OFFS[l], DS[l] * DS[l]
                nc.scalar.mul(
                    d_pre[:, off : off + blk],
                    d_f32[:, off : off + blk],
                    1.0 / (64.0 * np.sqrt(DS[l])),
                )

            # ---- sync-queue DMAs: x XBARs (l desc, split for pipelining) ----
            for l in LORDER:
                d = DS[l]
                nch = d * (d + 1) // 2
                # split into pieces of <= ~24 chunks at m boundaries
                per_m = (d + 1) // 2
                mstep = max(1, 24 // per_m)
                m0 = 0
                while m0 < d:
                    m1 = min(d, m0 + mstep)
                    c0, c1 = m0 * per_m, m1 * per_m
                    nc.sync.dma_start(
                        xt[l][:, c0:c1, :],
                        x_d[:, XOFF[l] + c0 * 128 : XOFF[l] + c1 * 128],
                        transpose=True,
                    )
                    m0 = m1

            # ---- psi: matmul into psiT chunks, park to DRAM, read back ----
            eng_flip = 0
            for ci, (r0, r1) in enumerate(PSI_CHUNKS):
                rows = r1 - r0
                psiT = psp.tile([128, F * F], BF, tag="psiT", name=f"psiT{ci}")
                for s in range(8):
                    pps = pa.tile([128, 512], F32, tag="pps", name=f"pps{ci}_{s}")
                    nc.tensor.matmul(
                        pps[:rows, :],
                        d_pre[:, r0:r1],
                        wT[:NROT, s * 512 : (s + 1) * 512],
                        start=True,
                        stop=True,
                    )
                    dst = psiT[:rows, s * 512 : (s + 1) * 512]
                    if eng_flip % 2 == 0:
                        nc.vector.tensor_copy(dst, pps[:rows, :])
                    else:
                        nc.scalar.copy(dst, pps[:rows, :])
                    eng_flip += 1
                nc.gpsimd.dma_start(s_d[ci][:, :], psiT[:rows, :])
                # read back every (l, u) fully contained in this chunk
                sv = s_d[ci].rearrange("i (f g) -> f i g", g=64)
                for l in LORDER:
                    d = DS[l]
                    off = OFFS[l]
                    for u in range(d):
                        ur0 = off + u * d
                        if ur0 < r0 or ur0 >= r1:
                            continue
                        ku, uin = divmod(u, 2)
                        dst = rhs[l][
                            uin * 64 : (uin + 1) * 64,
                            ku * d * 64 : (ku + 1) * d * 64,
                        ].rearrange("f (v g) -> f v g", g=64)
                        nc.gpsimd.dma_start(dst, sv[:, ur0 - r0 : ur0 - r0 + d, :])

            # ---- main loop ----
            for l in LORDER:
                d = DS[l]
                blk = d * d
                nku = (d + 1) // 2
                per_m = nku
                if d * 64 <= 512:
                    vsplits = [(0, d)]
                else:
                    vsplits = [(0, 8), (8, d - 8)]
                yv = yb[l].rearrange("b (g v m) -> b g v m", g=64, v=d)
                for m in range(d):
                    for v0, nv in vsplits:
                        pyt = py.tile([BS, 512], F32, tag="py", name=f"py{l}_{m}_{v0}")
                        out = pyt[:, : nv * 64]
                        for ku in range(nku):
                            c = m * per_m + ku
                            klast = (2 * ku + 1) >= d  # odd tail: real K=64
                            kk = 64 if klast else 128
                            nc.tensor.matmul(
                                out,
                                xt[l][:kk, c, :],
                                rhs[l][
                                    :kk,
                                    ku * d * 64 + v0 * 64 : ku * d * 64 + (v0 + nv) * 64,
                                ],
                                start=(ku == 0),
                                stop=(ku == nku - 1),
                            )
                        dst = yv[:, :, v0 : v0 + nv, m]
                        src = out.rearrange("b (v g) -> b g v", g=64)
                        if (m + v0) % 2 == 0:
                            nc.scalar.copy(dst, src)
                        else:
                            nc.vector.tensor_copy(dst, src)
                # store this l's y region (bf16, contiguous)
                nc.gpsimd.dma_start(
                    y_d[:, YOFF[l] : YOFF[l] + YLEN[l]], yb[l][:, :]
                )

    nc.compile()
    return nc


def _get_nc():
    if "nc" not in _CACHE:
        _CACHE["nc"] = _build()
    return _CACHE["nc"]


def _prep_x(xc):
    """[BS, F, IRREP] fp32 -> [BS, XTOT] bf16 in per-l (m, u-pad, f) layout."""
    import ml_dtypes

    out = np.zeros((BS, XTOT), dtype=ml_dtypes.bfloat16)
    for l in LORDER:
        d = DS[l]
        off = OFFS[l]
        xl = xc[:, :, off : off + d * d].reshape(BS, F, d, d)  # [b, f, u, m]
        arr = np.zeros((BS, d, d + 1, F), dtype=np.float32)  # [b, m, u-pad, f]
        arr[:, :, :d, :] = xl.transpose(0, 3, 2, 1)
        out[:, XOFF[l] : XOFF[l] + XLEN[l]] = (
            arr.reshape(BS, XLEN[l]).astype(ml_dtypes.bfloat16)
        )
    return out


def kernel(x, D, w):
    import ml_dtypes
    from concourse.bass_utils import run_bass_kernel_spmd

    nc = _get_nc()
    w2 = np.zeros((F * F, 128), dtype=ml_dtypes.bfloat16)
    w2[:, :NROT] = (
        np.asarray(w, dtype=np.float32)
        .reshape(F * F, NROT)
        .astype(ml_dtypes.bfloat16)
    )
    Dc = np.ascontiguousarray(np.asarray(D, dtype=np.float32))
    in_maps = [
        {
            "x4": _prep_x(np.asarray(x[c * BS : (c + 1) * BS], dtype=np.float32)),
            "w2": w2,
            "D": Dc,
        }
        for c in range(NCORES)
    ]
    res = run_bass_kernel_spmd(nc, in_maps, core_ids=list(range(NCORES)))
    yflat = np.concatenate(
        [r["y"].astype(np.float32) for r in res.results], axis=0
    )  # [B, YTOT]
    y = np.empty((B, F, IRREP), dtype=np.float32)
    for l in LORDER:
        d = DS[l]
        blk = d * d
        y[:, :, OFFS[l] : OFFS[l] + blk] = yflat[
            :, YOFF[l] : YOFF[l] + YLEN[l]
        ].reshape(B, F, blk)
    return y


# revision 5
# speedup vs baseline: 1.4538x; 1.2263x over previous
"""SO3Conv Trainium2 Bass kernel.

Math (per reference):
  psi[f,g,i] = sum_n D[n,i] w[f,g,n] / sqrt(64)
  per l (d=2l+1, blk=d*d at offset off):
    y[b,g,off+v*d+m] = 1/sqrt(64*d) * sum_{f,u} x[b,f,off+u*d+m] * psi[f,g,off+u*d+v]

Strategy: data-parallel over batch (8 cores x 128 batch).
Per core:
  A) x is pre-permuted on the host into per-l regions [b, (m, u-pad, f)]
     bf16 (u padded to d+1 slots).  XBAR DMA-transposes (InstDmaTransposeAnt)
     produce the matmul lhsT tiles [(u,f)-part, b-free] directly from DRAM.
  B) wT [n, (f g)] via one XBAR from host-padded w2 [(f g), n-pad].
  C) psi computed on PE in psiT layout [i-chunk-part, (f g)-free] (D
     pre-scaled per l on device), parked in DRAM scratch split at
     u-boundaries (one tensor per park piece for exact DMA deps), read back
     with (f,v,g) APs into per-l rhs tiles [(u-pair,f)-part, ku:(v,g)-free].
  D) main matmuls run ku-outer over m-groups of 4 (8 PSUM banks) so the PE
     consumes psi readbacks as they stream in; PSUM [b,(v g)] fp32 copied
     (cast bf16) into per-l y tiles in natural [b, g, v*d+m] order, stored
     bf16 to per-l DRAM regions; host converts to fp32 and reassembles.
  All DMAs ride HWDGE queues (sync = x XBARs + psi park/readback in priority
  order; scalar = D, wT, y stores) -- SWDGE's ~1.1us/DMA desc-gen is avoided.
"""

import sys

sys.path.insert(0, "/opt/trn_rl_repo")

import numpy as np

LMAX = 6
F = 64
NROT = 64
IRREP = 455
B = 1024
NCORES = 8
BS = B // NCORES  # 128

DS = [2 * l + 1 for l in range(LMAX + 1)]
OFFS = []
_o = 0
for _d in DS:
    OFFS.append(_o)
    _o += _d * _d
assert _o == IRREP

LORDER = list(range(LMAX, -1, -1))  # process l descending

# x4 DRAM region offsets (l descending), cols per l = d*(d+1)*64
XLEN = {l: DS[l] * (DS[l] + 1) * 64 for l in LORDER}
XOFF = {}
_o = 0
for l in LORDER:
    XOFF[l] = _o
    _o += XLEN[l]
XTOT = _o  # 32256

# y DRAM region offsets (l descending), cols per l = 64*blk
YLEN = {l: 64 * DS[l] * DS[l] for l in LORDER}
YOFF = {}
_o = 0
for l in LORDER:
    YOFF[l] = _o
    _o += YLEN[l]
YTOT = _o  # 29120

# psi matmul chunks: contiguous i-ranges, <=128 rows, l=6 first.
PSI_CHUNKS = [
    (OFFS[6], OFFS[6] + 9 * 13),       # c0: l6 u0..8   (117 rows)
    (OFFS[6] + 9 * 13, IRREP),         # c1: l6 u9..12  (52 rows)
    (OFFS[5], OFFS[6]),                # c2: l5         (121 rows)
    (OFFS[4], OFFS[5]),                # c3: l4         (81 rows)
    (0, OFFS[4]),                      # c4: l0..l3     (84 rows)
]
# park pieces (subranges of chunks, split at u-boundaries ~64 rows), each its
# own DRAM scratch tensor so readback deps are exact.
PARK = {
    0: [(286, 338), (338, 403)],       # l6 u0..3 / u4..8
    1: [(403, 455)],                   # l6 u9..12
    2: [(165, 231), (231, 286)],       # l5 u0..5 / u6..10
    3: [(84, 129), (129, 165)],        # l4 u0..4 / u5..8
    4: [(0, 35), (35, 84)],            # l0..l2 / l3
}

_CACHE = {}


def _build():
    import concourse.bacc as bacc
    import concourse.bass as bass
    import concourse.mybir as mybir
    from concourse import tile

    dt = mybir.dt
    BF = dt.bfloat16
    F32 = dt.float32

    nc = bacc.Bacc("TRN2", target_bir_lowering=False, debug=False, num_devices=NCORES)

    x_d = nc.dram_tensor("x4", [BS, XTOT], BF, kind="ExternalInput")
    w_d = nc.dram_tensor("w2", [F * F, 128], BF, kind="ExternalInput")
    D_d = nc.dram_tensor("D", [NROT, IRREP], F32, kind="ExternalInput")
    y_d = nc.dram_tensor("y", [BS, YTOT], BF, kind="ExternalOutput")
    park_t = {}  # (r0, r1) -> dram tensor
    for ci, pieces in PARK.items():
        for (r0, r1) in pieces:
            park_t[(r0, r1)] = nc.dram_tensor(
                f"psiS_{r0}_{r1}", [r1 - r0, F * F], BF
            )

    eng_flip = [0]

    with tile.TileContext(nc) as tc:
        with (
            tc.tile_pool(name="const", bufs=1) as cp,
            tc.tile_pool(name="xt", bufs=1) as xp,
            tc.tile_pool(name="rhs", bufs=1) as rp,
            tc.tile_pool(name="yb", bufs=1) as yp,
            tc.tile_pool(name="psit", bufs=3) as psp,
        ):
            # ---- persistent tiles ----
            wT = cp.tile([128, F * F], BF)
            d_f32 = cp.tile([NROT, IRREP], F32)
            d_pre = cp.tile([NROT, IRREP], BF)
            xt = {}   # per l: [128, nchunk, 128]; chunk c = m*nku + ku
            rhs = {}  # per l: [128, nku*d*64]; ku slab cols [ku*d*64, ...)
            yb = {}   # per l: [128, 64*blk]
            for l in LORDER:
                d = DS[l]
                xt[l] = xp.tile(
                    [128, d * (d + 1) // 2, 128], BF, name=f"xt{l}", tag=f"xt{l}"
                )
                rhs[l] = rp.tile(
                    [128, ((d + 1) // 2) * d * 64], BF, name=f"rhs{l}", tag=f"rhs{l}"
                )
                yb[l] = yp.tile([BS, 64 * d * d], BF, name=f"yb{l}", tag=f"yb{l}")

            # ---- scalar queue: D load, wT XBAR; then per-l D scaling ----
            nc.scalar.dma_start(d_f32[:, :], D_d[:, :])
            nc.scalar.dma_start(wT[:, :], w_d[:, :], transpose=True)
            for l in LORDER:
                off, blk = OFFS[l], DS[l] * DS[l]
                nc.scalar.mul(
                    d_pre[:, off : off + blk],
                    d_f32[:, off : off + blk],
                    1.0 / (64.0 * np.sqrt(DS[l])),
                )

            # ---- emission helpers ----
            def xbar_piece(l, m0, m1):
                nku = (DS[l] + 1) // 2
                c0, c1 = m0 * nku, m1 * nku
                nc.sync.dma_start(
                    xt[l][:, c0:c1, :],
                    x_d[:, XOFF[l] + c0 * 128 : XOFF[l] + c1 * 128],
                    transpose=True,
                )

            def xbar_l(l, groups):
                d = DS[l]
                for m0 in range(0, d, groups):
                    xbar_piece(l, m0, min(d, m0 + groups))

            psiT_tiles = {}

            def psi_mm(ci):
                r0, r1 = PSI_CHUNKS[ci]
                rows = r1 - r0
                psiT = psp.tile([128, F * F], BF, tag="psiT", name=f"psiT{ci}")
                psiT_tiles[ci] = psiT
                with tc.tile_pool(
                    name=f"pa{ci}", bufs=2, space=bass.MemorySpace.PSUM
                ) as pa:
                    for s in range(8):
                        pps = pa.tile(
                            [128, 512], F32, tag="pps", name=f"pps{ci}_{s}"
                        )
                        nc.tensor.matmul(
                            pps[:rows, :],
                            d_pre[:, r0:r1],
                            wT[:NROT, s * 512 : (s + 1) * 512],
                            start=True,
                            stop=True,
                        )
                        dst = psiT[:rows, s * 512 : (s + 1) * 512]
                        if eng_flip[0] % 2 == 0:
                            nc.vector.tensor_copy(dst, pps[:rows, :])
                        else:
                            nc.scalar.copy(dst, pps[:rows, :])
                        eng_flip[0] += 1

            def park(ci, pi):
                cr0, _ = PSI_CHUNKS[ci]
                r0, r1 = PARK[ci][pi]
                t = park_t[(r0, r1)]
                nc.sync.dma_start(
                    t[:, :], psiT_tiles[ci][r0 - cr0 : r1 - cr0, :]
                )

            def rb(l, u):
                d = DS[l]
                ur0 = OFFS[l] + u * d
                for (r0, r1), t in park_t.items():
                    if r0 <= ur0 and ur0 + d <= r1:
                        break
                else:
                    raise AssertionError((l, u))
                ku, uin = divmod(u, 2)
                dst = rhs[l][
                    uin * 64 : (uin + 1) * 64,
                    ku * d * 64 : (ku + 1) * d * 64,
                ].rearrange("f (v g) -> f v g", g=64)
                sv = t.rearrange("i (f g) -> f i g", g=64)
                nc.sync.dma_start(dst, sv[:, ur0 - r0 : ur0 - r0 + d, :])

            # ---- orchestrated prologue (sync-queue FIFO priority order) ----
            xbar_piece(6, 0, 4)                 # x6 m0-3
            psi_mm(0)
            park(0, 0)                          # l6 u0..3
            for u in range(0, 4):
                rb(6, u)
            park(0, 1)                          # l6 u4..8
            psi_mm(1)
            park(1, 0)                          # l6 u9..12
            for u in range(4, 13):
                rb(6, u)
            xbar_piece(6, 4, 8)
            xbar_piece(6, 8, 12)
            xbar_piece(6, 12, 13)
            psi_mm(2)
            xbar_l(5, 4)
            park(2, 0)
            park(2, 1)
            for u in range(11):
                rb(5, u)
            psi_mm(3)
            xbar_l(4, 4)
            park(3, 0)
            park(3, 1)
            for u in range(9):
                rb(4, u)
            psi_mm(4)
            for l in (3, 2, 1, 0):
                xbar_l(l, 8)
            park(4, 0)
            park(4, 1)
            for l in (3, 2, 1, 0):
                for u in range(DS[l]):
                    rb(l, u)

            # ---- main loop ----
            with tc.tile_pool(
                name="py", bufs=1, space=bass.MemorySpace.PSUM
            ) as py:
                for l in LORDER:
                    d = DS[l]
                    nku = (d + 1) // 2
                    if d * 64 <= 512:
                        vsplits = [(0, d)]
                    else:
                        vsplits = [(0, 8), (8, d - 8)]
                    mg_size = 4 if len(vsplits) == 2 else 8
                    yv = yb[l].rearrange("b (g v m) -> b g v m", g=64, v=d)
                    for mg0 in range(0, d, mg_size):
                        ms = list(range(mg0, min(d, mg0 + mg_size)))
                        pyt = {}
                        for m in ms:
                            for vi, (v0, nv) in enumerate(vsplits):
                                slot = (m - mg0) * len(vsplits) + vi
                                pyt[(m, v0)] = py.tile(
                                    [BS, 512], F32, tag=f"py{slot}",
                                    name=f"py{l}_{m}_{v0}",
                                )
                        for ku in range(nku):
                            kk = 64 if (2 * ku + 1) >= d else 128
                            for m in ms:
                                c = m * nku + ku
                                for (v0, nv) in vsplits:
                                    nc.tensor.matmul(
                                        pyt[(m, v0)][:, : nv * 64],
                                        xt[l][:kk, c, :],
                                        rhs[l][
                                            :kk,
                                            ku * d * 64
                                            + v0 * 64 : ku * d * 64
                                            + (v0 + nv) * 64,
                                        ],
                                        start=(ku == 0),
                                        stop=(ku == nku - 1),
                                    )
                        for m in ms:
                            for (v0, nv) in vsplits:
                                dst = yv[:, :, v0 : v0 + nv, m]
                                src = pyt[(m, v0)][:, : nv * 64].rearrange(
                                    "b (v g) -> b g v", g=64
                                )
                                if eng_flip[0] % 2 == 0:
                                    nc.scalar.copy(dst, src)
                                else:
                                    nc.vector.tensor_copy(dst, src)
                                eng_flip[0] += 1
                    nc.scalar.dma_start(
                        y_d[:, YOFF[l] : YOFF[l] + YLEN[l]], yb[l][:, :]
                    )

    nc.compile()
    return nc


def _get_nc():
    if "nc" not in _CACHE:
        _CACHE["nc"] = _build()
    return _CACHE["nc"]


def _prep_x(xc):
    """[BS, F, IRREP] fp32 -> [BS, XTOT] bf16 in per-l (m, u-pad, f) layout."""
    import ml_dtypes

    out = np.zeros((BS, XTOT), dtype=ml_dtypes.bfloat16)
    for l in LORDER:
        d = DS[l]
        off = OFFS[l]
        xl = xc[:, :, off : off + d * d].reshape(BS, F, d, d)  # [b, f, u, m]
        arr = np.zeros((BS, d, d + 1, F), dtype=np.float32)  # [b, m, u-pad, f]
        arr[:, :, :d, :] = xl.transpose(0, 3, 2, 1)
        out[:, XOFF[l] : XOFF[l] + XLEN[l]] = (
            arr.reshape(BS, XLEN[l]).astype(ml_dtypes.bfloat16)
        )
    return out


def kernel(x, D, w):
    import ml_dtypes
    from concourse.bass_utils import run_bass_kernel_spmd

    nc = _get_nc()
    w2 = np.zeros((F * F, 128), dtype=ml_dtypes.bfloat16)
    w2[:, :NROT] = (
        np.asarray(w, dtype=np.float32)
        .reshape(F * F, NROT)
        .astype(ml_dtypes.bfloat16)
    )
    Dc = np.ascontiguousarray(np.asarray(D, dtype=np.float32))
    in_maps = [
        {
            "x4": _prep_x(np.asarray(x[c * BS : (c + 1) * BS], dtype=np.float32)),
            "w2": w2,
            "D": Dc,
        }
        for c in range(NCORES)
    ]
    res = run_bass_kernel_spmd(nc, in_maps, core_ids=list(range(NCORES)))
    yflat = np.concatenate(
        [r["y"].astype(np.float32) for r in res.results], axis=0
    )  # [B, YTOT]
    y = np.empty((B, F, IRREP), dtype=np.float32)
    for l in LORDER:
        d = DS[l]
        blk = d * d
        y[:, :, OFFS[l] : OFFS[l] + blk] = yflat[
            :, YOFF[l] : YOFF[l] + YLEN[l]
        ].reshape(B, F, blk)
    return y


# revision 11
# speedup vs baseline: 1.6502x; 1.1351x over previous
"""SO3Conv Trainium2 Bass kernel.

Math (per reference):
  psi[f,g,i] = sum_n D[n,i] w[f,g,n] / sqrt(64)
  per l (d=2l+1, blk=d*d at offset off):
    y[b,g,off+v*d+m] = 1/sqrt(64*d) * sum_{f,u} x[b,f,off+u*d+m] * psi[f,g,off+u*d+v]

Strategy: data-parallel over batch (8 cores x 128 batch).
Per core:
  A) x is pre-permuted on the host into per-l regions [b, (m, u-pad, f)]
     bf16 (u padded to d+1 slots).  XBAR DMA-transposes (InstDmaTransposeAnt)
     produce the matmul lhsT tiles [(u,f)-part, b-free] directly from DRAM.
  B) wT [n, (f g)] via two XBARs from host-padded w2 [(f g), n-pad].
  C) psi computed on PE in psiT layout [i-chunk-part, (f g)-free] (D
     pre-scaled per l on device), parked in DRAM scratch (one tensor per
     chunk), read back into per-l rhs tiles [(u-pair,f)-part, ku:(v,g)-free]
     -- per-u for l6 (fine-grained early feed), per-u-parity for l<6.
  D) main matmuls run ku-outer over m-groups (8 PSUM banks) so the PE
     consumes psi readbacks as they stream in; PSUM [b,(v g)] fp32 copied
     (cast bf16) into per-l y tiles in natural [b, g, v*d+m] order, stored
     bf16 to per-l DRAM regions; host converts to fp32 and reassembles.
  DMA queues: sync carries the latency-critical chain in FIFO priority order
  (D, wT, x-l6, psi parks + readbacks, remaining x); scalar carries y stores.
"""

import sys

sys.path.insert(0, "/opt/trn_rl_repo")

import numpy as np

LMAX = 6
F = 64
NROT = 64
IRREP = 455
B = 1024
NCORES = 8
BS = B // NCORES  # 128

DS = [2 * l + 1 for l in range(LMAX + 1)]
OFFS = []
_o = 0
for _d in DS:
    OFFS.append(_o)
    _o += _d * _d
assert _o == IRREP

LORDER = list(range(LMAX, -1, -1))  # process l descending

# x4 DRAM region offsets (l descending), cols per l = d*(d+1)*64
XLEN = {l: DS[l] * (DS[l] + 1) * 64 for l in LORDER}
XOFF = {}
_o = 0
for l in LORDER:
    XOFF[l] = _o
    _o += XLEN[l]
XTOT = _o  # 32256

# y DRAM region offsets (l descending), cols per l = 64*blk
YLEN = {l: 64 * DS[l] * DS[l] for l in LORDER}
YOFF = {}
_o = 0
for l in LORDER:
    YOFF[l] = _o
    _o += YLEN[l]
YTOT = _o  # 29120

# psi matmul chunks: contiguous i-ranges, <=128 rows, l=6 first; one DRAM
# scratch tensor per chunk.
PSI_CHUNKS = [
    (OFFS[6], OFFS[6] + 9 * 13),       # c0: l6 u0..8   (117 rows)
    (OFFS[6] + 9 * 13, IRREP),         # c1: l6 u9..12  (52 rows)
    (OFFS[5], OFFS[6]),                # c2: l5         (121 rows)
    (OFFS[4], OFFS[5]),                # c3: l4         (81 rows)
    (0, OFFS[4]),                      # c4: l0..l3     (84 rows)
]

_CACHE = {}


def _build():
    import concourse.bacc as bacc
    import concourse.bass as bass
    import concourse.mybir as mybir
    from concourse import tile

    dt = mybir.dt
    BF = dt.bfloat16
    F32 = dt.float32

    nc = bacc.Bacc("TRN2", target_bir_lowering=False, debug=False, num_devices=NCORES)

    x_d = nc.dram_tensor("x4", [BS, XTOT], BF, kind="ExternalInput")
    w_d = nc.dram_tensor("w2", [F * F, 128], BF, kind="ExternalInput")
    D_d = nc.dram_tensor("D", [NROT, IRREP], F32, kind="ExternalInput")
    y_d = nc.dram_tensor("y", [BS, YTOT], BF, kind="ExternalOutput")
    # rows padded so rb_par's "(i2 j)" split (j=2d) divides evenly for every
    # l read from the chunk; pad rows are never written or read.
    PADROWS = {0: 117, 1: 52, 2: 132, 3: 90, 4: 210}
    park_t = [
        nc.dram_tensor(f"psiS{ci}", [PADROWS[ci], F * F], BF)
        for ci in range(len(PSI_CHUNKS))
    ]

    eng_flip = [0]

    with tile.TileContext(nc) as tc:
        with (
            tc.tile_pool(name="const", bufs=1) as cp,
            tc.tile_pool(name="xt", bufs=1) as xp,
            tc.tile_pool(name="rhs", bufs=1) as rp,
            tc.tile_pool(name="yb", bufs=1) as yp,
            tc.tile_pool(name="psit", bufs=3) as psp,
        ):
            # ---- persistent tiles ----
            wT = cp.tile([128, F * F], BF)
            d_f32 = cp.tile([NROT, IRREP], F32)
            d_pre = cp.tile([NROT, IRREP], BF)
            xt = {}   # l>=4: [128, nchunk, 128]; 'sm' = l3..l0 combined
            rhs = {}  # per l: [128, nku*d*64]; ku slab cols [ku*d*64, ...)
            yb = {}   # l>=4 per l; 'sm' combined for l3..l0
            for l in (6, 5, 4):
                d = DS[l]
                xt[l] = xp.tile(
                    [128, d * (d + 1) // 2, 128], BF, name=f"xt{l}", tag=f"xt{l}"
                )
                yb[l] = yp.tile([BS, 64 * d * d], BF, name=f"yb{l}", tag=f"yb{l}")
            NSM = sum(DS[l] * (DS[l] + 1) // 2 for l in (3, 2, 1, 0))  # 50
            xt["sm"] = xp.tile([128, NSM, 128], BF, name="xtsm", tag="xtsm")
            CB = {}  # chunk base within xt['sm']
            _c = 0
            for l in (3, 2, 1, 0):
                CB[l] = _c
                _c += DS[l] * (DS[l] + 1) // 2
            YSM = sum(YLEN[l] for l in (3, 2, 1, 0))  # 5376
            yb["sm"] = yp.tile([BS, YSM], BF, name="ybsm", tag="ybsm")
            YB = {l: YOFF[l] - YOFF[3] for l in (3, 2, 1, 0)}
            for l in LORDER:
                d = DS[l]
                rhs[l] = rp.tile(
                    [128, ((d + 1) // 2) * d * 64], BF, name=f"rhs{l}", tag=f"rhs{l}"
                )

            # ---- emission helpers ----
            def xbar(l, c0, c1):
                t = xt[l] if l in xt else xt["sm"]
                nc.sync.dma_start(
                    t[:, c0:c1, :],
                    x_d[:, XOFF[l] + c0 * 128 : XOFF[l] + c1 * 128]
                    if l != "sm"
                    else x_d[:, XOFF[3] + c0 * 128 : XOFF[3] + c1 * 128],
                    transpose=True,
                )

            psiT_tiles = {}

            def park(ci):
                r0, r1 = PSI_CHUNKS[ci]
                nc.sync.dma_start(
                    park_t[ci][: r1 - r0, :], psiT_tiles[ci][: r1 - r0, :]
                )

            def rb_u(l, u):
                """Per-u readback (l6 path)."""
                d = DS[l]
                ur0 = OFFS[l] + u * d
                for ci, (r0, r1) in enumerate(PSI_CHUNKS):
                    if r0 <= ur0 and ur0 + d <= r1:
                        break
                ku, uin = divmod(u, 2)
                dst = rhs[l][
                    uin * 64 : (uin + 1) * 64,
                    ku * d * 64 : (ku + 1) * d * 64,
                ].rearrange("f (v g) -> f v g", g=64)
                sv = park_t[ci].rearrange("i (f g) -> f i g", g=64)
                nc.sync.dma_start(dst, sv[:, ur0 - r0 : ur0 - r0 + d, :])

            def rb_par(l, uin):
                """Per-u-parity readback: all ku slabs of one parity at once."""
                d = DS[l]
                nk = (d - uin + 1) // 2  # number of u's with this parity
                u0row = OFFS[l] + uin * d  # first row of u=uin
                for ci, (r0, r1) in enumerate(PSI_CHUNKS):
                    if r0 <= u0row and OFFS[l] + d * d <= r1:
                        break
                else:
                    raise AssertionError((l, uin))
                dst = rhs[l][
                    uin * 64 : (uin + 1) * 64, : nk * d * 64
                ].rearrange("f (ku v g) -> f ku v g", ku=nk, g=64)
                sv = park_t[ci].rearrange("i (f g) -> f i g", g=64).rearrange(
                    "f (i2 j) g -> f i2 j g", j=2 * d
                )
                # rows for parity uin, ku: (2*ku+uin)*d + v  = ku*(2d) + uin*d + v
                base = u0row - r0
                i2_0 = base // (2 * d)
                voff = base % (2 * d)
                src = sv[:, i2_0 : i2_0 + nk, voff : voff + d, :]
                nc.sync.dma_start(dst, src)

            # ---- scalar-queue: D scaling; Act/DVE used for copies ----
            # sync queue, FIFO priority order:
            nc.sync.dma_start(d_f32[:, :], D_d[:, :])
            nc.sync.dma_start(wT[:, : 2048], w_d[:2048, :], transpose=True)
            nc.sync.dma_start(wT[:, 2048:], w_d[2048:, :], transpose=True)
            for l in LORDER:
                off, blk = OFFS[l], DS[l] * DS[l]
                nc.scalar.mul(
                    d_pre[:, off : off + blk],
                    d_f32[:, off : off + blk],
                    1.0 / (64.0 * np.sqrt(DS[l])),
                )

            def psi_mm(ci, pa):
                r0, r1 = PSI_CHUNKS[ci]
                rows = r1 - r0
                psiT = psp.tile([128, F * F], BF, tag="psiT", name=f"psiT{ci}")
                psiT_tiles[ci] = psiT
                for p in range(4):
                    pps = pa.tile([128, 1024], F32, tag="pps", name=f"pps{ci}_{p}")
                    for h in range(2):
                        s = 2 * p + h
                        nc.tensor.matmul(
                            pps[:rows, h * 512 : (h + 1) * 512],
                            d_pre[:, r0:r1],
                            wT[:NROT, s * 512 : (s + 1) * 512],
                            start=True,
                            stop=True,
                        )
                    dst = psiT[:rows, p * 1024 : (p + 1) * 1024]
                    if eng_flip[0] % 2 == 0:
                        nc.vector.tensor_copy(dst, pps[:rows, :])
                    else:
                        nc.scalar.copy(dst, pps[:rows, :])
                    eng_flip[0] += 1

            # ---- orchestrated prologue ----
            with tc.tile_pool(
                name="pa", bufs=3, space=bass.MemorySpace.PSUM
            ) as pa:
                xbar(6, 0, 28)            # l6 m0-3
                psi_mm(0, pa)
                park(0)
                for u in range(0, 9):
                    rb_u(6, u)
                psi_mm(1, pa)
                park(1)
                for u in range(9, 13):
                    rb_u(6, u)
                xbar(6, 28, 56)           # l6 m4-7
                xbar(6, 56, 91)           # l6 m8-12
                psi_mm(2, pa)
                xbar(5, 0, 48)            # l5 m0-7
                xbar(5, 48, 66)           # l5 m8-10
                park(2)
                for u in range(11):
                    rb_u(5, u)
                psi_mm(3, pa)
                xbar(4, 0, 45)
                park(3)
                for u in range(9):
                    rb_u(4, u)
                psi_mm(4, pa)
                xbar("sm", 0, NSM)
                park(4)
                for l in (3, 2, 1, 0):
                    for u in range(DS[l]):
                        rb_u(l, u)

            # ---- main loop ----
            with tc.tile_pool(
                name="py", bufs=1, space=bass.MemorySpace.PSUM
            ) as py:
                for l in LORDER:
                    d = DS[l]
                    nku = (d + 1) // 2
                    if d * 64 <= 512:
                        vsplits = [(0, d)]
                    else:
                        vsplits = [(0, 8), (8, d - 8)]
                    mg_size = 4 if len(vsplits) == 2 else 8
                    xtile = xt[l] if l >= 4 else xt["sm"]
                    cbase = 0 if l >= 4 else CB[l]
                    ytile = yb[l] if l >= 4 else yb["sm"]
                    ybase = 0 if l >= 4 else YB[l]
                    yv = ytile[:, ybase : ybase + 64 * d * d].rearrange(
                        "b (g v m) -> b g v m", g=64, v=d
                    )
                    for mg0 in range(0, d, mg_size):
                        ms = list(range(mg0, min(d, mg0 + mg_size)))
                        pyt = {}
                        for m in ms:
                            for vi, (v0, nv) in enumerate(vsplits):
                                slot = (m - mg0) * len(vsplits) + vi
                                pyt[(m, v0)] = py.tile(
                                    [BS, 512], F32, tag=f"py{slot}",
                                    name=f"py{l}_{m}_{v0}",
                                )
                        for ku in range(nku):
                            kk = 64 if (2 * ku + 1) >= d else 128
                            for m in ms:
                                c = cbase + m * nku + ku
                                for (v0, nv) in vsplits:
                                    nc.tensor.matmul(
                                        pyt[(m, v0)][:, : nv * 64],
                                        xtile[:kk, c, :],
                                        rhs[l][
                                            :kk,
                                            ku * d * 64
                                            + v0 * 64 : ku * d * 64
                                            + (v0 + nv) * 64,
                                        ],
                                        start=(ku == 0),
                                        stop=(ku == nku - 1),
                                    )
                        for m in ms:
                            for (v0, nv) in vsplits:
                                dst = yv[:, :, v0 : v0 + nv, m]
                                src = pyt[(m, v0)][:, : nv * 64].rearrange(
                                    "b (v g) -> b g v", g=64
                                )
                                if eng_flip[0] % 2 == 0:
                                    nc.scalar.copy(dst, src)
                                else:
                                    nc.vector.tensor_copy(dst, src)
                                eng_flip[0] += 1
                    if l >= 4:
                        nc.scalar.dma_start(
                            y_d[:, YOFF[l] : YOFF[l] + YLEN[l]], yb[l][:, :]
                        )
                    elif l == 0:
                        nc.scalar.dma_start(
                            y_d[:, YOFF[3] : YOFF[3] + YSM], yb["sm"][:, :]
                        )

    nc.compile()
    return nc


def _get_nc():
    if "nc" not in _CACHE:
        _CACHE["nc"] = _build()
    return _CACHE["nc"]


def _prep_x(xc):
    """[BS, F, IRREP] fp32 -> [BS, XTOT] bf16 in per-l (m, u-pad, f) layout."""
    import ml_dtypes

    out = np.zeros((BS, XTOT), dtype=ml_dtypes.bfloat16)
    for l in LORDER:
        d = DS[l]
        off = OFFS[l]
        xl = xc[:, :, off : off + d * d].reshape(BS, F, d, d)  # [b, f, u, m]
        arr = np.zeros((BS, d, d + 1, F), dtype=np.float32)  # [b, m, u-pad, f]
        arr[:, :, :d, :] = xl.transpose(0, 3, 2, 1)
        out[:, XOFF[l] : XOFF[l] + XLEN[l]] = (
            arr.reshape(BS, XLEN[l]).astype(ml_dtypes.bfloat16)
        )
    return out


def kernel(x, D, w):
    import ml_dtypes
    from concourse.bass_utils import run_bass_kernel_spmd

    nc = _get_nc()
    w2 = np.zeros((F * F, 128), dtype=ml_dtypes.bfloat16)
    w2[:, :NROT] = (
        np.asarray(w, dtype=np.float32)
        .reshape(F * F, NROT)
        .astype(ml_dtypes.bfloat16)
    )
    Dc = np.ascontiguousarray(np.asarray(D, dtype=np.float32))
    in_maps = [
        {
            "x4": _prep_x(np.asarray(x[c * BS : (c + 1) * BS], dtype=np.float32)),
            "w2": w2,
            "D": Dc,
        }
        for c in range(NCORES)
    ]
    res = run_bass_kernel_spmd(nc, in_maps, core_ids=list(range(NCORES)))
    yflat = np.concatenate(
        [r["y"].astype(np.float32) for r in res.results], axis=0
    )  # [B, YTOT]
    y = np.empty((B, F, IRREP), dtype=np.float32)
    for l in LORDER:
        d = DS[l]
        blk = d * d
        y[:, :, OFFS[l] : OFFS[l] + blk] = yflat[
            :, YOFF[l] : YOFF[l] + YLEN[l]
        ].reshape(B, F, blk)
    return y


# revision 12
# speedup vs baseline: 1.7357x; 1.0518x over previous
"""SO3Conv Trainium2 Bass kernel.

Math (per reference):
  psi[f,g,i] = sum_n D[n,i] w[f,g,n] / sqrt(64)
  per l (d=2l+1, blk=d*d at offset off):
    y[b,g,off+v*d+m] = 1/sqrt(64*d) * sum_{f,u} x[b,f,off+u*d+m] * psi[f,g,off+u*d+v]

Strategy: data-parallel over batch (8 cores x 128 batch).
Per core:
  A) x is pre-permuted on the host into per-l regions [b, (m, u-pad, f)]
     bf16 (u padded to d+1 slots).  XBAR DMA-transposes (InstDmaTransposeAnt)
     produce the matmul lhsT tiles [(u,f)-part, b-free] directly from DRAM.
  B) wT [n, (f g)] via two XBARs from host-padded w2 [(f g), n-pad].
  C) psi computed on PE in psiT layout [i-chunk-part, (f g)-free] (D
     pre-scaled per l on device), parked in DRAM scratch (one tensor per
     chunk), read back into per-l rhs tiles [(u-pair,f)-part, ku:(v,g)-free]
     -- per-u for l6 (fine-grained early feed), per-u-parity for l<6.
  D) main matmuls run ku-outer over m-groups (8 PSUM banks) so the PE
     consumes psi readbacks as they stream in; PSUM [b,(v g)] fp32 copied
     (cast bf16) into per-l y tiles in natural [b, g, v*d+m] order, stored
     bf16 to per-l DRAM regions; host converts to fp32 and reassembles.
  DMA queues: sync carries the latency-critical chain in FIFO priority order
  (D, wT, x-l6, psi parks + readbacks, remaining x); scalar carries y stores.
"""

import sys

sys.path.insert(0, "/opt/trn_rl_repo")

import numpy as np

LMAX = 6
F = 64
NROT = 64
IRREP = 455
B = 1024
NCORES = 8
BS = B // NCORES  # 128

DS = [2 * l + 1 for l in range(LMAX + 1)]
OFFS = []
_o = 0
for _d in DS:
    OFFS.append(_o)
    _o += _d * _d
assert _o == IRREP

LORDER = list(range(LMAX, -1, -1))  # process l descending

# x4 DRAM region offsets (l descending), cols per l = d*(d+1)*64
XLEN = {l: DS[l] * (DS[l] + 1) * 64 for l in LORDER}
XOFF = {}
_o = 0
for l in LORDER:
    XOFF[l] = _o
    _o += XLEN[l]
XTOT = _o  # 32256

# y DRAM region offsets (l descending), cols per l = 64*blk
YLEN = {l: 64 * DS[l] * DS[l] for l in LORDER}
YOFF = {}
_o = 0
for l in LORDER:
    YOFF[l] = _o
    _o += YLEN[l]
YTOT = _o  # 29120

# psi matmul chunks: contiguous i-ranges, <=128 rows, l=6 first; one DRAM
# scratch tensor per chunk.
PSI_CHUNKS = [
    (OFFS[6], OFFS[6] + 9 * 13),       # c0: l6 u0..8   (117 rows)
    (OFFS[6] + 9 * 13, IRREP),         # c1: l6 u9..12  (52 rows)
    (OFFS[5], OFFS[6]),                # c2: l5         (121 rows)
    (OFFS[4], OFFS[5]),                # c3: l4         (81 rows)
    (0, OFFS[4]),                      # c4: l0..l3     (84 rows)
]

_CACHE = {}


def _build():
    import concourse.bacc as bacc
    import concourse.bass as bass
    import concourse.mybir as mybir
    from concourse import tile

    dt = mybir.dt
    BF = dt.bfloat16
    F32 = dt.float32

    nc = bacc.Bacc("TRN2", target_bir_lowering=False, debug=False, num_devices=NCORES)

    x_d = nc.dram_tensor("x4", [BS, XTOT], BF, kind="ExternalInput")
    w_d = nc.dram_tensor("w2", [F * F, 128], BF, kind="ExternalInput")
    D_d = nc.dram_tensor("D", [NROT, IRREP], F32, kind="ExternalInput")
    y_d = nc.dram_tensor("y", [BS, YTOT], BF, kind="ExternalOutput")
    # rows padded so rb_par's "(i2 j)" split (j=2d) divides evenly for every
    # l read from the chunk; pad rows are never written or read.
    PADROWS = {0: 117, 1: 52, 2: 132, 3: 90, 4: 210}
    park_t = [
        nc.dram_tensor(f"psiS{ci}", [PADROWS[ci], F * F], BF)
        for ci in range(len(PSI_CHUNKS))
    ]

    eng_flip = [0]

    with tile.TileContext(nc) as tc:
        with (
            tc.tile_pool(name="const", bufs=1) as cp,
            tc.tile_pool(name="xt", bufs=1) as xp,
            tc.tile_pool(name="rhs", bufs=1) as rp,
            tc.tile_pool(name="yb", bufs=1) as yp,
            tc.tile_pool(name="psit", bufs=4) as psp,
        ):
            # ---- persistent tiles ----
            wT = cp.tile([128, F * F], BF)
            d_f32 = cp.tile([NROT, IRREP], F32)
            d_pre = cp.tile([NROT, IRREP], BF)
            xt = {}   # l>=4: [128, nchunk, 128]; 'sm' = l3..l0 combined
            rhs = {}  # per l: [128, nku*d*64]; ku slab cols [ku*d*64, ...)
            yb = {}   # l>=4 per l; 'sm' combined for l3..l0
            for l in (6, 5, 4):
                d = DS[l]
                xt[l] = xp.tile(
                    [128, d * (d + 1) // 2, 128], BF, name=f"xt{l}", tag=f"xt{l}"
                )
                yb[l] = yp.tile([BS, 64 * d * d], BF, name=f"yb{l}", tag=f"yb{l}")
            NSM = sum(DS[l] * (DS[l] + 1) // 2 for l in (3, 2, 1, 0))  # 50
            xt["sm"] = xp.tile([128, NSM, 128], BF, name="xtsm", tag="xtsm")
            CB = {}  # chunk base within xt['sm']
            _c = 0
            for l in (3, 2, 1, 0):
                CB[l] = _c
                _c += DS[l] * (DS[l] + 1) // 2
            YSM = sum(YLEN[l] for l in (3, 2, 1, 0))  # 5376
            yb["sm"] = yp.tile([BS, YSM], BF, name="ybsm", tag="ybsm")
            YB = {l: YOFF[l] - YOFF[3] for l in (3, 2, 1, 0)}
            for l in LORDER:
                d = DS[l]
                rhs[l] = rp.tile(
                    [128, ((d + 1) // 2) * d * 64], BF, name=f"rhs{l}", tag=f"rhs{l}"
                )

            # ---- emission helpers ----
            def xbar(l, c0, c1):
                t = xt[l] if l in xt else xt["sm"]
                nc.sync.dma_start(
                    t[:, c0:c1, :],
                    x_d[:, XOFF[l] + c0 * 128 : XOFF[l] + c1 * 128]
                    if l != "sm"
                    else x_d[:, XOFF[3] + c0 * 128 : XOFF[3] + c1 * 128],
                    transpose=True,
                )

            psiT_tiles = {}

            def park(ci):
                r0, r1 = PSI_CHUNKS[ci]
                nc.sync.dma_start(
                    park_t[ci][: r1 - r0, :], psiT_tiles[ci][: r1 - r0, :]
                )

            def rb_u(l, u):
                """Per-u readback (l6 path)."""
                d = DS[l]
                ur0 = OFFS[l] + u * d
                for ci, (r0, r1) in enumerate(PSI_CHUNKS):
                    if r0 <= ur0 and ur0 + d <= r1:
                        break
                ku, uin = divmod(u, 2)
                dst = rhs[l][
                    uin * 64 : (uin + 1) * 64,
                    ku * d * 64 : (ku + 1) * d * 64,
                ].rearrange("f (v g) -> f v g", g=64)
                sv = park_t[ci].rearrange("i (f g) -> f i g", g=64)
                nc.sync.dma_start(dst, sv[:, ur0 - r0 : ur0 - r0 + d, :])

            def rb_par(l, uin):
                """Per-u-parity readback: all ku slabs of one parity at once."""
                d = DS[l]
                nk = (d - uin + 1) // 2  # number of u's with this parity
                u0row = OFFS[l] + uin * d  # first row of u=uin
                for ci, (r0, r1) in enumerate(PSI_CHUNKS):
                    if r0 <= u0row and OFFS[l] + d * d <= r1:
                        break
                else:
                    raise AssertionError((l, uin))
                dst = rhs[l][
                    uin * 64 : (uin + 1) * 64, : nk * d * 64
                ].rearrange("f (ku v g) -> f ku v g", ku=nk, g=64)
                sv = park_t[ci].rearrange("i (f g) -> f i g", g=64).rearrange(
                    "f (i2 j) g -> f i2 j g", j=2 * d
                )
                # rows for parity uin, ku: (2*ku+uin)*d + v  = ku*(2d) + uin*d + v
                base = u0row - r0
                i2_0 = base // (2 * d)
                voff = base % (2 * d)
                src = sv[:, i2_0 : i2_0 + nk, voff : voff + d, :]
                nc.sync.dma_start(dst, src)

            # ---- scalar-queue: D scaling; Act/DVE used for copies ----
            # sync queue, FIFO priority order:
            nc.sync.dma_start(d_f32[:, :], D_d[:, :])
            nc.sync.dma_start(wT[:, : 2048], w_d[:2048, :], transpose=True)
            nc.sync.dma_start(wT[:, 2048:], w_d[2048:, :], transpose=True)
            for l in LORDER:
                off, blk = OFFS[l], DS[l] * DS[l]
                nc.scalar.mul(
                    d_pre[:, off : off + blk],
                    d_f32[:, off : off + blk],
                    1.0 / (64.0 * np.sqrt(DS[l])),
                )

            def psi_mm(ci, pa):
                r0, r1 = PSI_CHUNKS[ci]
                rows = r1 - r0
                psiT = psp.tile([128, F * F], BF, tag="psiT", name=f"psiT{ci}")
                psiT_tiles[ci] = psiT
                for p in range(4):
                    pps = pa.tile([128, 1024], F32, tag="pps", name=f"pps{ci}_{p}")
                    for h in range(2):
                        s = 2 * p + h
                        nc.tensor.matmul(
                            pps[:rows, h * 512 : (h + 1) * 512],
                            d_pre[:, r0:r1],
                            wT[:NROT, s * 512 : (s + 1) * 512],
                            start=True,
                            stop=True,
                        )
                    dst = psiT[:rows, p * 1024 : (p + 1) * 1024]
                    if eng_flip[0] % 2 == 0:
                        nc.vector.tensor_copy(dst, pps[:rows, :])
                    else:
                        nc.scalar.copy(dst, pps[:rows, :])
                    eng_flip[0] += 1

            # ---- orchestrated prologue ----
            with tc.tile_pool(
                name="pa", bufs=3, space=bass.MemorySpace.PSUM
            ) as pa:
                xbar(6, 0, 28)            # l6 m0-3
                psi_mm(0, pa)
                park(0)
                for u in range(0, 9):
                    rb_u(6, u)
                psi_mm(1, pa)
                park(1)
                for u in range(9, 13):
                    rb_u(6, u)
                xbar(6, 28, 56)           # l6 m4-7
                xbar(6, 56, 91)           # l6 m8-12
                psi_mm(2, pa)
                xbar(5, 0, 48)            # l5 m0-7
                xbar(5, 48, 66)           # l5 m8-10
                park(2)
                for u in range(11):
                    rb_u(5, u)
                psi_mm(3, pa)
                xbar(4, 0, 45)
                park(3)
                for u in range(9):
                    rb_u(4, u)
                psi_mm(4, pa)
                xbar("sm", 0, NSM)
                park(4)
                for l in (3, 2, 1, 0):
                    for u in range(DS[l]):
                        rb_u(l, u)

            # ---- main loop ----
            with tc.tile_pool(
                name="py", bufs=1, space=bass.MemorySpace.PSUM
            ) as py:
                for l in LORDER:
                    d = DS[l]
                    nku = (d + 1) // 2
                    if d * 64 <= 512:
                        vsplits = [(0, d)]
                    else:
                        vsplits = [(0, 8), (8, d - 8)]
                    mg_size = 4 if len(vsplits) == 2 else 8
                    xtile = xt[l] if l >= 4 else xt["sm"]
                    cbase = 0 if l >= 4 else CB[l]
                    ytile = yb[l] if l >= 4 else yb["sm"]
                    ybase = 0 if l >= 4 else YB[l]
                    yv = ytile[:, ybase : ybase + 64 * d * d].rearrange(
                        "b (g v m) -> b g v m", g=64, v=d
                    )
                    for mg0 in range(0, d, mg_size):
                        ms = list(range(mg0, min(d, mg0 + mg_size)))
                        pyt = {}
                        for m in ms:
                            for vi, (v0, nv) in enumerate(vsplits):
                                slot = (m - mg0) * len(vsplits) + vi
                                pyt[(m, v0)] = py.tile(
                                    [BS, 512], F32, tag=f"py{slot}",
                                    name=f"py{l}_{m}_{v0}",
                                )
                        for ku in range(nku):
                            kk = 64 if (2 * ku + 1) >= d else 128
                            for m in ms:
                                c = cbase + m * nku + ku
                                for (v0, nv) in vsplits:
                                    nc.tensor.matmul(
                                        pyt[(m, v0)][:, : nv * 64],
                                        xtile[:kk, c, :],
                                        rhs[l][
                                            :kk,
                                            ku * d * 64
                                            + v0 * 64 : ku * d * 64
                                            + (v0 + nv) * 64,
                                        ],
                                        start=(ku == 0),
                                        stop=(ku == nku - 1),
                                    )
                        for m in ms:
                            for (v0, nv) in vsplits:
                                dst = yv[:, :, v0 : v0 + nv, m]
                                src = pyt[(m, v0)][:, : nv * 64].rearrange(
                                    "b (v g) -> b g v", g=64
                                )
                                if eng_flip[0] % 2 == 0:
                                    nc.scalar.copy(dst, src)
                                else:
                                    nc.vector.tensor_copy(dst, src)
                                eng_flip[0] += 1
                    if l >= 4:
                        nc.sync.dma_start(
                            y_d[:, YOFF[l] : YOFF[l] + YLEN[l]], yb[l][:, :]
                        )
                    elif l == 2:
                        cut = YB[1]
                        nc.sync.dma_start(
                            y_d[:, YOFF[3] : YOFF[3] + cut], yb["sm"][:, :cut]
                        )
                    elif l == 0:
                        cut = YB[1]
                        nc.sync.dma_start(
                            y_d[:, YOFF[3] + cut : YOFF[3] + YSM],
                            yb["sm"][:, cut:],
                        )

    nc.compile()
    return nc


def _get_nc():
    if "nc" not in _CACHE:
        _CACHE["nc"] = _build()
    return _CACHE["nc"]


def _prep_x(xc):
    """[BS, F, IRREP] fp32 -> [BS, XTOT] bf16 in per-l (m, u-pad, f) layout."""
    import ml_dtypes

    out = np.zeros((BS, XTOT), dtype=ml_dtypes.bfloat16)
    for l in LORDER:
        d = DS[l]
        off = OFFS[l]
        xl = xc[:, :, off : off + d * d].reshape(BS, F, d, d)  # [b, f, u, m]
        arr = np.zeros((BS, d, d + 1, F), dtype=np.float32)  # [b, m, u-pad, f]
        arr[:, :, :d, :] = xl.transpose(0, 3, 2, 1)
        out[:, XOFF[l] : XOFF[l] + XLEN[l]] = (
            arr.reshape(BS, XLEN[l]).astype(ml_dtypes.bfloat16)
        )
    return out


def kernel(x, D, w):
    import ml_dtypes
    from concourse.bass_utils import run_bass_kernel_spmd

    nc = _get_nc()
    w2 = np.zeros((F * F, 128), dtype=ml_dtypes.bfloat16)
    w2[:, :NROT] = (
        np.asarray(w, dtype=np.float32)
        .reshape(F * F, NROT)
        .astype(ml_dtypes.bfloat16)
    )
    Dc = np.ascontiguousarray(np.asarray(D, dtype=np.float32))
    in_maps = [
        {
            "x4": _prep_x(np.asarray(x[c * BS : (c + 1) * BS], dtype=np.float32)),
            "w2": w2,
            "D": Dc,
        }
        for c in range(NCORES)
    ]
    res = run_bass_kernel_spmd(nc, in_maps, core_ids=list(range(NCORES)))
    yflat = np.concatenate(
        [r["y"].astype(np.float32) for r in res.results], axis=0
    )  # [B, YTOT]
    y = np.empty((B, F, IRREP), dtype=np.float32)
    for l in LORDER:
        d = DS[l]
        blk = d * d
        y[:, :, OFFS[l] : OFFS[l] + blk] = yflat[
            :, YOFF[l] : YOFF[l] + YLEN[l]
        ].reshape(B, F, blk)
    return y


# revision 14
# speedup vs baseline: 1.7940x; 1.0336x over previous
"""SO3Conv Trainium2 Bass kernel.

Math (per reference):
  psi[f,g,i] = sum_n D[n,i] w[f,g,n] / sqrt(64)
  per l (d=2l+1, blk=d*d at offset off):
    y[b,g,off+v*d+m] = 1/sqrt(64*d) * sum_{f,u} x[b,f,off+u*d+m] * psi[f,g,off+u*d+v]

Strategy: data-parallel over batch (8 cores x 128 batch).
Per core:
  A) x is pre-permuted on the host into per-l regions [b, (m, u-pad, f)]
     bf16 (u padded to d+1 slots).  XBAR DMA-transposes (InstDmaTransposeAnt)
     produce the matmul lhsT tiles [(u,f)-part, b-free] directly from DRAM.
  B) wT [n, (f g)] via two XBARs from host-padded w2 [(f g), n-pad].
  C) psi computed on PE in psiT layout [i-chunk-part, (f g)-free] (D
     pre-scaled per l on device), parked in DRAM scratch (one tensor per
     chunk), read back into per-l rhs tiles [(u-pair,f)-part, ku:(v,g)-free]
     -- per-u for l6 (fine-grained early feed), per-u-parity for l<6.
  D) main matmuls run ku-outer over m-groups (8 PSUM banks) so the PE
     consumes psi readbacks as they stream in; PSUM [b,(v g)] fp32 copied
     (cast bf16) into per-l y tiles in natural [b, g, v*d+m] order, stored
     bf16 to per-l DRAM regions; host converts to fp32 and reassembles.
  DMA queues: sync carries the latency-critical chain in FIFO priority order
  (D, wT, x-l6, psi parks + readbacks, remaining x); scalar carries y stores.
"""

import sys

sys.path.insert(0, "/opt/trn_rl_repo")

import numpy as np

LMAX = 6
F = 64
NROT = 64
IRREP = 455
B = 1024
NCORES = 8
BS = B // NCORES  # 128

DS = [2 * l + 1 for l in range(LMAX + 1)]
OFFS = []
_o = 0
for _d in DS:
    OFFS.append(_o)
    _o += _d * _d
assert _o == IRREP

LORDER = list(range(LMAX, -1, -1))  # process l descending

# x4 DRAM region offsets (l descending), cols per l = d*(d+1)*64
XLEN = {l: DS[l] * (DS[l] + 1) * 64 for l in LORDER}
XOFF = {}
_o = 0
for l in LORDER:
    XOFF[l] = _o
    _o += XLEN[l]
XTOT = _o  # 32256

# y DRAM region offsets (l descending), cols per l = 64*blk
YLEN = {l: 64 * DS[l] * DS[l] for l in LORDER}
YOFF = {}
_o = 0
for l in LORDER:
    YOFF[l] = _o
    _o += YLEN[l]
YTOT = _o  # 29120

# psi matmul chunks: contiguous i-ranges, <=128 rows, l=6 first; one DRAM
# scratch tensor per chunk.
PSI_CHUNKS = [
    (OFFS[6], OFFS[6] + 9 * 13),       # c0: l6 u0..8   (117 rows)
    (OFFS[6] + 9 * 13, IRREP),         # c1: l6 u9..12  (52 rows)
    (OFFS[5], OFFS[6]),                # c2: l5         (121 rows)
    (OFFS[4], OFFS[5]),                # c3: l4         (81 rows)
    (0, OFFS[4]),                      # c4: l0..l3     (84 rows)
]

_CACHE = {}


def _build():
    import concourse.bacc as bacc
    import concourse.bass as bass
    import concourse.mybir as mybir
    from concourse import tile

    dt = mybir.dt
    BF = dt.bfloat16
    F32 = dt.float32

    nc = bacc.Bacc("TRN2", target_bir_lowering=False, debug=False, num_devices=NCORES)

    x_d = nc.dram_tensor("x4", [BS, XTOT], BF, kind="ExternalInput")
    w_d = nc.dram_tensor("w2", [F * F, 128], BF, kind="ExternalInput")
    D_d = nc.dram_tensor("D", [NROT, IRREP], F32, kind="ExternalInput")
    y_d = nc.dram_tensor("y", [BS, YTOT], BF, kind="ExternalOutput")
    # rows padded so rb_par's "(i2 j)" split (j=2d) divides evenly for every
    # l read from the chunk; pad rows are never written or read.
    PADROWS = {0: 117, 1: 52, 2: 132, 3: 90, 4: 210}
    park_t = [
        nc.dram_tensor(f"psiS{ci}", [PADROWS[ci], F * F], BF)
        for ci in range(len(PSI_CHUNKS))
    ]

    eng_flip = [0]

    with tile.TileContext(nc) as tc:
        with (
            tc.tile_pool(name="const", bufs=1) as cp,
            tc.tile_pool(name="xt", bufs=1) as xp,
            tc.tile_pool(name="rhs", bufs=1) as rp,
            tc.tile_pool(name="yb", bufs=1) as yp,
            tc.tile_pool(name="psit", bufs=4) as psp,
        ):
            # ---- persistent tiles ----
            wT = cp.tile([128, F * F], BF)
            d_f32 = cp.tile([NROT, IRREP], F32)
            d_pre = cp.tile([NROT, IRREP], BF)
            xt = {}   # l>=4: [128, nchunk, 128]; 'sm' = l3..l0 combined
            rhs = {}  # per l: [128, nku*d*64]; ku slab cols [ku*d*64, ...)
            yb = {}   # l>=4 per l; 'sm' combined for l3..l0
            for l in (6, 5, 4):
                d = DS[l]
                xt[l] = xp.tile(
                    [128, d * (d + 1) // 2, 128], BF, name=f"xt{l}", tag=f"xt{l}"
                )
                yb[l] = yp.tile([BS, 64 * d * d], BF, name=f"yb{l}", tag=f"yb{l}")
            NSM = sum(DS[l] * (DS[l] + 1) // 2 for l in (3, 2, 1, 0))  # 50
            xt["sm"] = xp.tile([128, NSM, 128], BF, name="xtsm", tag="xtsm")
            CB = {}  # chunk base within xt['sm']
            _c = 0
            for l in (3, 2, 1, 0):
                CB[l] = _c
                _c += DS[l] * (DS[l] + 1) // 2
            YSM = sum(YLEN[l] for l in (3, 2, 1, 0))  # 5376
            yb["sm"] = yp.tile([BS, YSM], BF, name="ybsm", tag="ybsm")
            YB = {l: YOFF[l] - YOFF[3] for l in (3, 2, 1, 0)}
            for l in LORDER:
                d = DS[l]
                rhs[l] = rp.tile(
                    [128, ((d + 1) // 2) * d * 64], BF, name=f"rhs{l}", tag=f"rhs{l}"
                )

            # ---- emission helpers ----
            def xbar(l, c0, c1, q=None):
                t = xt[l] if l in xt else xt["sm"]
                (q or nc.sync).dma_start(
                    t[:, c0:c1, :],
                    x_d[:, XOFF[l] + c0 * 128 : XOFF[l] + c1 * 128]
                    if l != "sm"
                    else x_d[:, XOFF[3] + c0 * 128 : XOFF[3] + c1 * 128],
                    transpose=True,
                )

            psiT_tiles = {}

            def park(ci, q=None):
                r0, r1 = PSI_CHUNKS[ci]
                (q or nc.sync).dma_start(
                    park_t[ci][: r1 - r0, :], psiT_tiles[ci][: r1 - r0, :]
                )

            def rb_u(l, u, q=None):
                """Per-u readback."""
                d = DS[l]
                ur0 = OFFS[l] + u * d
                for ci, (r0, r1) in enumerate(PSI_CHUNKS):
                    if r0 <= ur0 and ur0 + d <= r1:
                        break
                ku, uin = divmod(u, 2)
                dst = rhs[l][
                    uin * 64 : (uin + 1) * 64,
                    ku * d * 64 : (ku + 1) * d * 64,
                ].rearrange("f (v g) -> f v g", g=64)
                sv = park_t[ci].rearrange("i (f g) -> f i g", g=64)
                (q or nc.sync).dma_start(dst, sv[:, ur0 - r0 : ur0 - r0 + d, :])

            def rb_par(l, uin):
                """Per-u-parity readback: all ku slabs of one parity at once."""
                d = DS[l]
                nk = (d - uin + 1) // 2  # number of u's with this parity
                u0row = OFFS[l] + uin * d  # first row of u=uin
                for ci, (r0, r1) in enumerate(PSI_CHUNKS):
                    if r0 <= u0row and OFFS[l] + d * d <= r1:
                        break
                else:
                    raise AssertionError((l, uin))
                dst = rhs[l][
                    uin * 64 : (uin + 1) * 64, : nk * d * 64
                ].rearrange("f (ku v g) -> f ku v g", ku=nk, g=64)
                sv = park_t[ci].rearrange("i (f g) -> f i g", g=64).rearrange(
                    "f (i2 j) g -> f i2 j g", j=2 * d
                )
                # rows for parity uin, ku: (2*ku+uin)*d + v  = ku*(2d) + uin*d + v
                base = u0row - r0
                i2_0 = base // (2 * d)
                voff = base % (2 * d)
                src = sv[:, i2_0 : i2_0 + nk, voff : voff + d, :]
                nc.sync.dma_start(dst, src)

            # sync queue, FIFO priority order:
            nc.sync.dma_start(wT[:, : 2048], w_d[:2048, :], transpose=True)
            nc.sync.dma_start(d_f32[:, :], D_d[:, :])
            nc.sync.dma_start(wT[:, 2048:], w_d[2048:, :], transpose=True)
            for l in LORDER:
                off, blk = OFFS[l], DS[l] * DS[l]
                nc.scalar.mul(
                    d_pre[:, off : off + blk],
                    d_f32[:, off : off + blk],
                    1.0 / (64.0 * np.sqrt(DS[l])),
                )

            def psi_mm(ci, pa):
                r0, r1 = PSI_CHUNKS[ci]
                rows = r1 - r0
                psiT = psp.tile([128, F * F], BF, tag="psiT", name=f"psiT{ci}")
                psiT_tiles[ci] = psiT
                for p in range(4):
                    pps = pa.tile([128, 1024], F32, tag="pps", name=f"pps{ci}_{p}")
                    for h in range(2):
                        s = 2 * p + h
                        nc.tensor.matmul(
                            pps[:rows, h * 512 : (h + 1) * 512],
                            d_pre[:, r0:r1],
                            wT[:NROT, s * 512 : (s + 1) * 512],
                            start=True,
                            stop=True,
                        )
                    dst = psiT[:rows, p * 1024 : (p + 1) * 1024]
                    if eng_flip[0] % 2 == 0:
                        nc.vector.tensor_copy(dst, pps[:rows, :])
                    else:
                        nc.scalar.copy(dst, pps[:rows, :])
                    eng_flip[0] += 1

            # ---- orchestrated prologue ----
            with tc.tile_pool(
                name="pa", bufs=3, space=bass.MemorySpace.PSUM
            ) as pa:
                # sync (SP): l6 chain
                xbar(6, 0, 56)            # l6 m0-7
                psi_mm(0, pa)
                park(0)
                for u in range(0, 9):
                    rb_u(6, u)
                psi_mm(1, pa)
                park(1)
                for u in range(9, 13):
                    rb_u(6, u)
                xbar(6, 56, 91)           # l6 m8-12
                psi_mm(2, pa)
                # sync continues: l5 chain (park2 waits psi copies, so the
                # x5/x4/xsm XBARs queue behind the l6-critical prologue)
                park(2)
                for u in range(11):
                    rb_u(5, u)
                psi_mm(3, pa)
                # gpsimd (Pool SWDGE, otherwise idle): l4 + l3..l0 chains
                park(3, nc.gpsimd)
                for u in range(9):
                    rb_u(4, u, nc.gpsimd)
                xbar(5, 0, 48)            # l5 m0-7
                xbar(5, 48, 66)
                xbar(4, 0, 45)
                psi_mm(4, pa)
                park(4, nc.gpsimd)
                for l in (3, 2, 1, 0):
                    for u in range(DS[l]):
                        rb_u(l, u, nc.gpsimd)
                xbar("sm", 0, NSM)

            # ---- main loop ----
            with tc.tile_pool(
                name="py", bufs=1, space=bass.MemorySpace.PSUM
            ) as py:
                for l in LORDER:
                    d = DS[l]
                    nku = (d + 1) // 2
                    if d * 64 <= 512:
                        vsplits = [(0, d)]
                    else:
                        vsplits = [(0, 8), (8, d - 8)]
                    mg_size = 4 if len(vsplits) == 2 else 8
                    xtile = xt[l] if l >= 4 else xt["sm"]
                    cbase = 0 if l >= 4 else CB[l]
                    ytile = yb[l] if l >= 4 else yb["sm"]
                    ybase = 0 if l >= 4 else YB[l]
                    yv = ytile[:, ybase : ybase + 64 * d * d].rearrange(
                        "b (g v m) -> b g v m", g=64, v=d
                    )
                    for mg0 in range(0, d, mg_size):
                        ms = list(range(mg0, min(d, mg0 + mg_size)))
                        pyt = {}
                        for m in ms:
                            for vi, (v0, nv) in enumerate(vsplits):
                                slot = (m - mg0) * len(vsplits) + vi
                                pyt[(m, v0)] = py.tile(
                                    [BS, 512], F32, tag=f"py{slot}",
                                    name=f"py{l}_{m}_{v0}",
                                )
                        for ku in range(nku):
                            kk = 64 if (2 * ku + 1) >= d else 128
                            for m in ms:
                                c = cbase + m * nku + ku
                                for (v0, nv) in vsplits:
                                    nc.tensor.matmul(
                                        pyt[(m, v0)][:, : nv * 64],
                                        xtile[:kk, c, :],
                                        rhs[l][
                                            :kk,
                                            ku * d * 64
                                            + v0 * 64 : ku * d * 64
                                            + (v0 + nv) * 64,
                                        ],
                                        start=(ku == 0),
                                        stop=(ku == nku - 1),
                                    )
                        for m in ms:
                            for (v0, nv) in vsplits:
                                dst = yv[:, :, v0 : v0 + nv, m]
                                src = pyt[(m, v0)][:, : nv * 64].rearrange(
                                    "b (v g) -> b g v", g=64
                                )
                                if eng_flip[0] % 2 == 0:
                                    nc.scalar.copy(dst, src)
                                else:
                                    nc.vector.tensor_copy(dst, src)
                                eng_flip[0] += 1
                    if l >= 4:
                        (nc.sync if l == 6 else nc.gpsimd).dma_start(
                            y_d[:, YOFF[l] : YOFF[l] + YLEN[l]], yb[l][:, :]
                        )
                    elif l == 2:
                        cut = YB[1]
                        nc.gpsimd.dma_start(
                            y_d[:, YOFF[3] : YOFF[3] + cut], yb["sm"][:, :cut]
                        )
                    elif l == 0:
                        cut = YB[1]
                        nc.gpsimd.dma_start(
                            y_d[:, YOFF[3] + cut : YOFF[3] + YSM],
                            yb["sm"][:, cut:],
                        )

    nc.compile()
    return nc


def _get_nc():
    if "nc" not in _CACHE:
        _CACHE["nc"] = _build()
    return _CACHE["nc"]


def _prep_x(xc):
    """[BS, F, IRREP] fp32 -> [BS, XTOT] bf16 in per-l (m, u-pad, f) layout."""
    import ml_dtypes

    out = np.zeros((BS, XTOT), dtype=ml_dtypes.bfloat16)
    for l in LORDER:
        d = DS[l]
        off = OFFS[l]
        xl = xc[:, :, off : off + d * d].reshape(BS, F, d, d)  # [b, f, u, m]
        arr = np.zeros((BS, d, d + 1, F), dtype=np.float32)  # [b, m, u-pad, f]
        arr[:, :, :d, :] = xl.transpose(0, 3, 2, 1)
        out[:, XOFF[l] : XOFF[l] + XLEN[l]] = (
            arr.reshape(BS, XLEN[l]).astype(ml_dtypes.bfloat16)
        )
    return out


def kernel(x, D, w):
    import ml_dtypes
    from concourse.bass_utils import run_bass_kernel_spmd

    nc = _get_nc()
    w2 = np.zeros((F * F, 128), dtype=ml_dtypes.bfloat16)
    w2[:, :NROT] = (
        np.asarray(w, dtype=np.float32)
        .reshape(F * F, NROT)
        .astype(ml_dtypes.bfloat16)
    )
    Dc = np.ascontiguousarray(np.asarray(D, dtype=np.float32))
    in_maps = [
        {
            "x4": _prep_x(np.asarray(x[c * BS : (c + 1) * BS], dtype=np.float32)),
            "w2": w2,
            "D": Dc,
        }
        for c in range(NCORES)
    ]
    res = run_bass_kernel_spmd(nc, in_maps, core_ids=list(range(NCORES)))
    yflat = np.concatenate(
        [r["y"].astype(np.float32) for r in res.results], axis=0
    )  # [B, YTOT]
    y = np.empty((B, F, IRREP), dtype=np.float32)
    for l in LORDER:
        d = DS[l]
        blk = d * d
        y[:, :, OFFS[l] : OFFS[l] + blk] = yflat[
            :, YOFF[l] : YOFF[l] + YLEN[l]
        ].reshape(B, F, blk)
    return y
